# revision 1
# baseline (speedup 1.0000x reference)
"""FISTA sparse-coding encoder kernel for Trainium2 (8 NeuronCores).

Problem: x [2,10,20480] f32, Drr/Dtheta [40] f32.
  D = normalized dictionary [10, 161]
  A = I - D^T D / L,  DtY = D^T Y / L,  lam = gamma / L
  40 FISTA iterations: xn = softshrink(A @ y + DtY); y = xn + m (xn - x_old)
  output sparsecode [2, 161, 20480].

Design ("v4"): data-parallel over columns (5120 per core), u-form momentum
  u_i = A x_i + DtY (augmented matmul with Y glued into the tail contraction),
  xn_{i+1} = shrink((1+m) u_i - m u_{i-1}, lam).

Per iteration:
  PE   : 8 matmuls per 1024-col group (4 passes x 512 free), f32r full rate.
  ACT + Pool: evacuate u PSUM -> SBUF as UNSCALED f32 copies ("uev").  Tail
         [33,512] chunks are packed two-per-tile at partition bases 0 and 64
         (ACT performs the partition-shifting copy), so the tail's DVE cost
         halves (cost on these engines is free-size based, partitions free).
  DVE  : fused clamp-form shrink with both momentum scales
         xn = w - clip(w, -lam, lam), w = s0*uev_cur + s1*uev_prev
         (s0 = 1+m, s1 = -m), all operands SBUF -> no PSUM access penalty.
  DMA  : SP + ACT hardware queues repack the packed tail xn back into the
         glued [43, NCOLS] tail state (partition-shifting SBUF->SBUF copies),
         and stream the final iteration's output to DRAM.

This moves the shrink pass off the critical PSUM path and cuts DVE busy time
from ~12.7us/iter (baseline, PSUM-sourced op + full-width tail) to ~8.5us,
leaving the PE's 8.5us/iter as the bottleneck.
"""

import numpy as np

# ---------------------------------------------------------------- constants
B, T, N_POLES, P = 2, 10, 40, 20480
MAX_ITER = 40
GAMMA = 0.01
K = 4 * N_POLES + 1          # 161
NCORES = 8
NCOLS = B * P // NCORES      # 5120 columns per core
BLK = 512                    # matmul free dim (one PSUM bank)
GRP = 1024                   # PSUM group (2 blocks)
NGRP = NCOLS // GRP          # 5
KH = 128                     # head rows
KT = K - KH                  # 33 tail rows
KA = KT + T                  # 43 = tail rows + glued Y rows
PACK = 97                    # packed tail partitions (33 @ base 0, 33 @ 64)
NPK = NCOLS // 2             # 2560 packed tail columns

_cache = {}


# ------------------------------------------------------------- custom DVE op
def _register_shrinkmom3():
    """out = w - clip(w, -C2, C2)  with  w = in0*s0 + in1*s1  (7 ALU stages).

    Equals softshrink(w, C2); both momentum scales live in the op, so the
    PSUM evacuation copies need no scaling engine.
    """
    import concourse.dve_ops as dve_ops
    from concourse.dve_spec import Spec, Src0, Src1, C0, C1, C2, Zero, maxx, minn, lower
    from concourse.dve_spec import _has_src1 as has_src1
    from concourse.dve_uop import DveOpSpec

    name = "ANT_SHRINKMOM3_FISTA"
    if any(op.name == name for op in dve_ops.OPS):
        return next(op for op in dve_ops.OPS if op.name == name)

    w = Src0 * C0 + Src1 * C1
    spec = Spec(
        body=w - minn(maxx(w, Zero - C2), C2),
        reference=lambda in0, in1, s0=1.0, s1=0.0, imm2=0.0: (
            lambda ww: (ww - np.minimum(np.maximum(ww, -imm2), imm2))
            .astype(np.float32)
        )(in0 * s0 + in1 * s1),
    )
    op = dve_ops.DveOp(name, spec, subdim=False, uops_sha={})
    dve_ops.OPS.append(op)
    dve_ops.CUSTOM_DVE_SPECS[name] = spec
    dve_ops._SUB_OPCODE_FOR_NAME[name] = (
        dve_ops._CUSTOM_DVE_ROW_BASE + len(dve_ops.OPS) - 1
    )
    for ver in ("v3", "v4"):
        compiled = DveOpSpec(
            name=name,
            opcode=dve_ops.get_dve_sub_opcode(name),
            uops=lower(spec, ver=ver),
            rd1_en=has_src1(spec),
        )
        op.uops_sha[ver] = compiled.sha(ver)
    return op


# ------------------------------------------------------------ host constants
def _host_constants(Drr, Dtheta):
    r = Drr.astype(np.float64)
    th = Dtheta.astype(np.float64)
    i = np.arange(T, dtype=np.float64)[:, None]
    pr = r[None, :] ** i
    sgn = np.where(np.arange(T)[:, None] % 2 == 0, 1.0, -1.0)
    c = np.cos(i * th[None, :])
    s = np.sin(i * th[None, :])
    ones = np.ones((T, 1))
    dic = np.concatenate([ones, pr * c, sgn * pr * c, pr * s, sgn * pr * s], axis=1)
    G = np.linalg.norm(dic, axis=0)
    G = np.where(G == 0, np.sqrt(float(T)), G)
    D = (dic / G).astype(np.float32)            # [T, K]

    D64 = D.astype(np.float64)
    DtD = D64.T @ D64
    L = float(np.linalg.norm(DtD))              # Frobenius
    A = np.eye(K) - DtD / L                     # [K, K]
    lam = float(GAMMA / L)

    Aaug = np.concatenate([A.T, D64 / L], axis=0).astype(np.float32)  # [171, K]

    # momentum coefficients m_i = (t_i - 1)/t_{i+1}, t_0 = 1
    ms = []
    t = 1.0
    for _ in range(MAX_ITER):
        t_new = (1.0 + np.sqrt(1.0 + 4.0 * t * t)) / 2.0
        ms.append((t - 1.0) / t_new)
        t = t_new
    return Aaug, lam, ms


# ------------------------------------------------------------- bass program
def _build_program():
    import concourse.mybir as mybir
    import concourse.tile as tile
    from concourse import bacc

    fused_op = _register_shrinkmom3()

    f32 = mybir.dt.float32
    f32r = mybir.dt.float32r

    nc = bacc.Bacc("TRN2", target_bir_lowering=False, debug=False,
                   num_devices=NCORES)

    ycols = nc.dram_tensor("ycols", [T, NCOLS], f32, kind="ExternalInput")
    d_l1a = nc.dram_tensor("l1a", [KH, KH], f32, kind="ExternalInput")
    d_l1b = nc.dram_tensor("l1b", [KH, KT], f32, kind="ExternalInput")
    d_l2a = nc.dram_tensor("l2a", [KA, KH], f32, kind="ExternalInput")
    d_l2b = nc.dram_tensor("l2b", [KA, KT], f32, kind="ExternalInput")
    d_l0a = nc.dram_tensor("l0a", [KA, KH], f32, kind="ExternalInput")
    d_l0b = nc.dram_tensor("l0b", [KA, KT], f32, kind="ExternalInput")
    out = nc.dram_tensor("out", [K, NCOLS], f32, kind="ExternalOutput")
    dbg = {}
    if _cache.get("debug_dumps"):
        for nm, shp in (("d_xh0", [KH, NCOLS]), ("d_xh1", [KH, NCOLS]),
                        ("d_xt0", [KA, NCOLS]), ("d_xt1", [KA, NCOLS]),
                        ("d_uevh0", [KH, NCOLS]), ("d_uevh1", [KH, NCOLS]),
                        ("d_uevt0", [PACK, NPK]), ("d_uevt1", [PACK, NPK]),
                        ("d_xpk0", [PACK, NPK]), ("d_xpk1", [PACK, NPK]),
                        ("d_ygl", [KA, NCOLS])):
            dbg[nm] = nc.dram_tensor(nm, shp, f32, kind="ExternalOutput")

    lam, ms = _cache["consts_meta"]

    with tile.TileContext(nc) as tc:
        with (
            tc.tile_pool(name="state", bufs=1) as st,
            tc.tile_pool(name="wts", bufs=1) as wts,
            tc.tile_pool(name="psH", bufs=2, space="PSUM") as psHp,
            tc.tile_pool(name="psT", bufs=4, space="PSUM") as psTp,
        ):
            # ---- persistent state -------------------------------------
            xH = [st.tile([KH, NCOLS], f32r, tag=f"xH{b}", name=f"xH{b}")
                  for b in range(2)]
            xT = [st.tile([KA, NCOLS], f32r, tag=f"xT{b}", name=f"xT{b}")
                  for b in range(2)]          # rows KT:KA hold glued Y
            uevH = [st.tile([KH, NCOLS], f32, tag=f"uevH{b}", name=f"uevH{b}")
                    for b in range(2)]
            uevT = [st.tile([PACK, NPK], f32, tag=f"uevT{b}", name=f"uevT{b}")
                    for b in range(2)]
            xPk = [st.tile([PACK, NPK], f32r, tag=f"xPk{b}", name=f"xPk{b}")
                   for b in range(2)]

            # fp32 staging for DMA'd weights -> rounded f32r copies
            lt1a = wts.tile([KH, KH], f32, tag="lt1a", name="lt1a")
            lt1b = wts.tile([KH, KT], f32, tag="lt1b", name="lt1b")
            lt2a = wts.tile([KA, KH], f32, tag="lt2a", name="lt2a")
            lt2b = wts.tile([KA, KT], f32, tag="lt2b", name="lt2b")
            l1a = wts.tile([KH, KH], f32r, tag="l1a", name="l1a")
            l1b = wts.tile([KH, KT], f32r, tag="l1b", name="l1b")
            l2a = wts.tile([KA, KH], f32r, tag="l2a", name="l2a")
            l2b = wts.tile([KA, KT], f32r, tag="l2b", name="l2b")
            lt0a = wts.tile([KA, KH], f32, tag="lt0a", name="lt0a")
            lt0b = wts.tile([KA, KT], f32, tag="lt0b", name="lt0b")
            l0a = wts.tile([KA, KH], f32r, tag="l0a", name="l0a")
            l0b = wts.tile([KA, KT], f32r, tag="l0b", name="l0b")
            # K=10 f32r matmuls crash the PE; pad the DtY contraction to 43
            # rows (Y on top, zeros below) — 43 is hardware-validated.
            ygl = wts.tile([KA, NCOLS], f32r, tag="ygl", name="ygl")

            nc.sync.dma_start(lt1a[:], d_l1a[:])
            nc.sync.dma_start(lt1b[:], d_l1b[:])
            nc.sync.dma_start(lt2a[:], d_l2a[:])
            nc.sync.dma_start(lt2b[:], d_l2b[:])
            nc.sync.dma_start(lt0a[:], d_l0a[:])
            nc.sync.dma_start(lt0b[:], d_l0b[:])
            nc.scalar.copy(l0a[:], lt0a[:])
            nc.scalar.copy(l0b[:], lt0b[:])
            nc.scalar.copy(l1a[:], lt1a[:])
            nc.scalar.copy(l1b[:], lt1b[:])
            nc.scalar.copy(l2a[:], lt2a[:])
            nc.scalar.copy(l2b[:], lt2b[:])

            # ---- init.  No state/uev memsets needed: iter 0 uses in1=in0
            # with s1=0, and the uevT pack holes (partitions 33..63) are
            # written-but-never-read.  f32r tiles need rounding producers,
            # so Y goes through an f32 staging tile and engine copies:
            # ygl rows 0..9 feed the it-0 DtY contraction, and the xT glue
            # rows 33..42 ride the tail contraction every iteration (row 32
            # zeroed here, overwritten each iteration by the tail repack).
            half = NCOLS // 2
            with tc.tile_pool(name="init", bufs=1) as ip:
                gstage = ip.tile([KA, NCOLS], f32, tag="gstage", name="gstage")
                # Column-pipelined init: each 1024-col chunk flows
                # memset -> Y DMA -> rounding copies independently, so the
                # it-0 matmuls start after the first chunk instead of after
                # a full-width serial chain.
                for c in range(NGRP):
                    cs = slice(c * GRP, (c + 1) * GRP)
                    q = nc.sync if c % 2 == 0 else nc.scalar
                    nc.gpsimd.memset(gstage[0:32, cs], 0.0)
                    nc.gpsimd.memset(gstage[32:KA, cs], 0.0)
                    q.dma_start(gstage[0:T, cs], ycols[:, cs])
                    q.dma_start(gstage[KT:KA, cs], ycols[:, cs])
                    nc.vector.tensor_copy(ygl[0:32, cs], gstage[0:32, cs])
                    nc.scalar.copy(ygl[32:KA, cs], gstage[32:KA, cs])
                    nc.gpsimd.tensor_copy(xT[1][32:KA, cs],
                                          gstage[32:KA, cs])
                    nc.gpsimd.tensor_copy(xT[0][32:KA, cs],
                                          gstage[32:KA, cs])

            def mm(ps, lhsT, rhs, start, stop):
                nc.tensor.matmul(ps, lhsT, rhs, start=start, stop=stop)

            for it in range(MAX_ITER):
                cur = it % 2
                nxt = (it + 1) % 2           # also the "previous" uev buffer
                m_prev = ms[it - 1] if it > 0 else 0.0
                s0 = float(1.0 + m_prev)
                s1 = float(-m_prev)

                # At it=0 s1=0 and the previous-u buffer is uninitialized, so
                # alias in1 to the current buffer (contributes s1*in1 = 0).
                h_prev = cur if it == 0 else nxt

                for g in range(NGRP):
                    gs = slice(g * GRP, (g + 1) * GRP)
                    pk = slice(g * BLK, (g + 1) * BLK)

                    wh = psHp.tile([KH, GRP], mybir.dt.float32, tag="wh",
                                   name="wh")
                    wtb = [psTp.tile([KT, BLK], mybir.dt.float32, tag="wt",
                                     name="wt") for _ in range(GRP // BLK)]

                    for b in range(GRP // BLK):
                        bs = slice(g * GRP + b * BLK, g * GRP + (b + 1) * BLK)
                        pb = slice(b * BLK, (b + 1) * BLK)
                        if it == 0:
                            mm(wh[:, pb], l0a[:], ygl[:, bs], True, True)
                        else:
                            mm(wh[:, pb], l1a[:], xH[cur][:, bs], True, False)
                            mm(wh[:, pb], l2a[:], xT[cur][:, bs], False, True)
                    for b in range(GRP // BLK):
                        bs = slice(g * GRP + b * BLK, g * GRP + (b + 1) * BLK)
                        pb = slice(b * BLK, (b + 1) * BLK)
                        if it == 0:
                            mm(wtb[b][:], l0b[:], ygl[:, bs], True, True)
                        else:
                            mm(wtb[b][:], l1b[:], xH[cur][:, bs], True, False)
                            mm(wtb[b][:], l2b[:], xT[cur][:, bs], False, True)

                    # ---- evacuate u (unscaled) PSUM -> SBUF f32, tail first.
                    # Only ACT and DVE can read PSUM on TRN2.  Tail block 1
                    # is partition-shifted to base 64 (ACT) -> packed
                    # [97, 512] per group; block 0 copies plain (ACT for
                    # g<3, DVE for the last two groups to balance load).
                    nc.scalar.copy(uevT[cur][64:64 + KT, pk], wtb[1][:])
                    if g >= 2:
                        nc.scalar.copy(uevT[cur][0:KT, pk], wtb[0][:])
                    else:
                        nc.vector.tensor_copy(uevT[cur][0:KT, pk], wtb[0][:])
                    if it < MAX_ITER - 1:
                        nc.scalar.copy(uevH[cur][:, gs], wh[:])

                    # ---- fused momentum + soft-threshold (all-SBUF),
                    # per group, tail before head (the tail feeds the
                    # repack -> next-iter PE chain).
                    nc.vector._custom_dve(
                        fused_op, out=xPk[cur][:, pk],
                        in0=uevT[cur][:, pk], in1=uevT[h_prev][:, pk],
                        s0=s0, s1=s1, imm2=float(lam))
                    h_in0 = (uevH[cur][:, gs] if it < MAX_ITER - 1
                             else wh[:])
                    nc.vector._custom_dve(
                        fused_op, out=xH[nxt][:, gs],
                        in0=h_in0, in1=uevH[h_prev][:, gs],
                        s0=s0, s1=s1, imm2=float(lam))

                    # ---- tail repack (packed [97, 512] -> glued [43, *]),
                    # or final output streaming.
                    if it < MAX_ITER - 1:
                        nc.sync.dma_start(
                            xT[nxt][0:KT, g * GRP:g * GRP + BLK],
                            xPk[cur][0:KT, pk])
                        nc.sync.dma_start(
                            xT[nxt][0:KT, g * GRP + BLK:(g + 1) * GRP],
                            xPk[cur][64:64 + KT, pk])
                    else:
                        f32c = mybir.dt.float32
                        nc.sync.dma_start(
                            out[KH:K, g * GRP:g * GRP + BLK],
                            xPk[cur][0:KT, pk].bitcast(f32c))
                        nc.sync.dma_start(
                            out[KH:K, g * GRP + BLK:(g + 1) * GRP],
                            xPk[cur][64:64 + KT, pk].bitcast(f32c))
                        nc.gpsimd.dma_start(out[0:KH, gs],
                                            xH[nxt][:, gs].bitcast(f32c))
            if dbg:
                f32c = mybir.dt.float32
                nc.sync.dma_start(dbg["d_xh0"][:], xH[0][:].bitcast(f32c))
                nc.sync.dma_start(dbg["d_xh1"][:], xH[1][:].bitcast(f32c))
                nc.sync.dma_start(dbg["d_xt0"][:], xT[0][:].bitcast(f32c))
                nc.sync.dma_start(dbg["d_xt1"][:], xT[1][:].bitcast(f32c))
                nc.sync.dma_start(dbg["d_uevh0"][:], uevH[0][:])
                nc.sync.dma_start(dbg["d_uevh1"][:], uevH[1][:])
                nc.sync.dma_start(dbg["d_uevt0"][:], uevT[0][:])
                nc.sync.dma_start(dbg["d_uevt1"][:], uevT[1][:])
                nc.sync.dma_start(dbg["d_xpk0"][:], xPk[0][:].bitcast(f32c))
                nc.sync.dma_start(dbg["d_xpk1"][:], xPk[1][:].bitcast(f32c))
                nc.sync.dma_start(dbg["d_ygl"][:], ygl[:].bitcast(f32c))
    nc.finalize()
    return nc


def _get_program(lam, ms):
    key = (round(lam, 12), tuple(round(m, 9) for m in ms))
    if _cache.get("key") != key:
        _cache["consts_meta"] = (lam, ms)
        _cache["nc"] = _build_program()
        _cache["key"] = key
    return _cache["nc"]


# ------------------------------------------------------------------- kernel
def kernel(x, Drr, Dtheta):
    from concourse.bass_utils import run_bass_kernel_spmd

    Aaug, lam, ms = _host_constants(Drr, Dtheta)
    nc = _get_program(lam, ms)

    l1a = np.ascontiguousarray(Aaug[0:KH, 0:KH])
    l1b = np.ascontiguousarray(Aaug[0:KH, KH:K])
    l2a = np.ascontiguousarray(Aaug[KH:KH + KA, 0:KH])
    l2b = np.ascontiguousarray(Aaug[KH:KH + KA, KH:K])
    l0a = np.zeros((KA, KH), np.float32)
    l0b = np.zeros((KA, KT), np.float32)
    l0a[0:T] = Aaug[K:K + T, 0:KH]
    l0b[0:T] = Aaug[K:K + T, KH:K]

    xc = np.ascontiguousarray(
        np.transpose(x.astype(np.float32), (1, 0, 2)).reshape(T, B * P))

    in_maps = []
    for c in range(NCORES):
        in_maps.append({
            "ycols": np.ascontiguousarray(xc[:, c * NCOLS:(c + 1) * NCOLS]),
            "l1a": l1a, "l1b": l1b, "l2a": l2a, "l2b": l2b,
            "l0a": l0a, "l0b": l0b,
        })

    res = run_bass_kernel_spmd(nc, in_maps, core_ids=list(range(NCORES)))
    _cache["last_res"] = res
    full = np.concatenate([r["out"] for r in res.results], axis=1)  # [K, B*P]
    return np.ascontiguousarray(
        full.reshape(K, B, P).transpose(1, 0, 2)).astype(np.float32)


if __name__ == "__main__":
    x = np.random.randn(B, T, P).astype(np.float32)
    Drr = np.random.rand(N_POLES).astype(np.float32)
    Dtheta = np.random.rand(N_POLES).astype(np.float32)
    o = kernel(x, Drr, Dtheta)
    print(o.shape, o.dtype)

